# revision 16
# baseline (speedup 1.0000x reference)
"""Banded causal self-attention (sparse_attention) for 8 trn2 NeuronCores.

Sharding: tensor-parallel over head groups (4 groups x 4 heads of dim 64)
x data-parallel over batch (2). Core c handles batch c//4, head group c%4.
Each core computes a partial output projection; the host sums the 4 group
partials per batch.

Layout: x is transposed on the host so every matmul on device uses natural
(pre-transposed) operands:
  qkT[512, T]   = W_qk.T @ x.T      (lhsT = W_qk natural, rhs = xT)
  v[T, 256]     = x @ W_v           (lhsT = xT natural,   rhs = W_v)
  scoresT[tk,tq]  computed as lhsT=kT rhs=qT  (both slices of qkT)
  yT+sums       = lhsT=[v|1] rhs=exp(scoresT)  (sums row = softmax denom)
  out[T, C]     = lhsT=yTpair rhs=W_p pair rows (K=128, 2 pairs)
Softmax skips max-subtraction (scores ~ N(0,1) after 1/8 scale; exp is safe
in fp32), so the partition-dim reduction is a fused ones-column in the
att@v matmul.

Head pairs (2h, 2h+1) sit at partition bases 0/64 of shared tiles, so score
matmuls for a pair occupy disjoint PE row groups and run concurrently, and
the projection contracts K=128 across a pair in one matmul.

dtype variants: "f32" (exact), "f32r" (tf32-like PE fast path), "bf16".
"""

import numpy as np

B, T, C = 2, 2048, 1024
N_HEAD = 16
MEMORY = 256
D = 64           # head dim
G = 4            # head groups (tensor parallel)
HPG = 4          # heads per group
GC = HPG * D     # 256 columns per group
N_CORES = 8
TB = T // 128    # 16 row blocks
SB = T // 256    # 8 query super-blocks

_PROGRAM_CACHE = {}


def _emit(tc, nc, xT, wqk, wv, wp, ones_in, out, dtype):
    import concourse.mybir as mybir

    f32 = mybir.dt.float32
    mmdt = {
        "f32": f32,
        "f32r": mybir.dt.float32r,
        "bf16": mybir.dt.bfloat16,
    }[dtype]
    # dtype of the yT/sums accumulator tile (needs f32-ish for reciprocal)
    ytdt = f32 if dtype == "bf16" else mmdt
    pool_masks = dtype != "f32r"   # affine_select can't write f32r

    from contextlib import ExitStack

    ctx = ExitStack()
    with ctx:
        const = ctx.enter_context(tc.tile_pool(name="const", bufs=1))
        wpool = ctx.enter_context(tc.tile_pool(name="wpool", bufs=1))
        arena = ctx.enter_context(tc.tile_pool(name="arena", bufs=9))
        qkt_pool = ctx.enter_context(tc.tile_pool(name="qkt", bufs=1))
        vplus_pool = ctx.enter_context(tc.tile_pool(name="vplus", bufs=1))
        expst_pool = ctx.enter_context(tc.tile_pool(name="expst", bufs=4))
        outsb_pool = ctx.enter_context(tc.tile_pool(name="outsb", bufs=3))
        ps1 = ctx.enter_context(tc.tile_pool(name="ps1", bufs=2, space="PSUM"))
        ps2 = ctx.enter_context(tc.tile_pool(name="ps2", bufs=3, space="PSUM"))

        # ---- constants / masks ----
        if not pool_masks:
            from concourse.masks import (
                make_lower_triangular,
                make_upper_triangular,
            )

            up_mask = const.tile([128, 128], f32, name="up_mask", tag="up_mask")
            make_upper_triangular(nc, up_mask[:], val=1.0, diag=True)  # 1 if p<=f
            lo_mask = const.tile([128, 128], f32, name="lo_mask", tag="lo_mask")
            make_lower_triangular(nc, lo_mask[:], val=1.0, diag=True)  # 1 if p>=f

        def mask_up(sl):
            # keep p <= f, else 0   (sl is a [128, 128] slice of expst)
            if pool_masks:
                nc.gpsimd.affine_select(
                    out=sl, in_=sl, compare_op=mybir.AluOpType.is_ge,
                    fill=0.0, base=0, pattern=[[1, 128]], channel_multiplier=-1,
                )
            else:
                nc.vector.tensor_mul(sl, sl, up_mask[:])

        def mask_lo(sl):
            # keep p >= f, else 0
            if pool_masks:
                nc.gpsimd.affine_select(
                    out=sl, in_=sl, compare_op=mybir.AluOpType.is_ge,
                    fill=0.0, base=0, pattern=[[-1, 128]], channel_multiplier=1,
                )
            else:
                nc.vector.tensor_mul(sl, sl, lo_mask[:])

        # ones are DMA'd from DRAM (memset can't produce f32r)
        ones_sb = const.tile([128, 64], mmdt, name="ones_sb", tag="ones_sb")
        nc.sync.dma_start(ones_sb[:], ones_in[:, 0:64])


        # ---- input loads (interleaved so phase A can start early) ----
        xT_sb, wqk_sb, wv_sb = [], [], []
        for k in range(8):
            qa = nc.sync if k % 2 == 0 else nc.scalar
            qb = nc.scalar if k % 2 == 0 else nc.sync
            t = arena.tile([128, T], mmdt, name=f"xT{k}", tag="arena")
            qa.dma_start(t[:], xT[k * 128:(k + 1) * 128, :])
            xT_sb.append(t)
            t = wpool.tile([128, 2 * GC], mmdt, name=f"wqk{k}", tag=f"wqk{k}")
            qb.dma_start(t[:], wqk[k * 128:(k + 1) * 128, :])
            wqk_sb.append(t)
            t = wpool.tile([128, GC], mmdt, name=f"wv{k}", tag=f"wv{k}")
            qb.dma_start(t[:], wv[k * 128:(k + 1) * 128, :])
            wv_sb.append(t)
        wp_sb = []
        for pr in range(2):
            t = wpool.tile([128, C], mmdt, name=f"wp{pr}", tag=f"wp{pr}")
            nc.gpsimd.dma_start(t[:], wp[pr * 128:(pr + 1) * 128, :])
            wp_sb.append(t)

        # warm up the PE clock gate during the input-load dead time: ~7us of
        # full-array matmuls on the first loaded tiles (result funneled to
        # out[0:1, 0:1] before phase E overwrites it, so nothing is DCE'd)
        warm_sb = const.tile([64, 512], f32, name="warm_sb", tag="warm_sb")
        warm_ps = ps2.tile([128, 512], f32, name="warm_ps", tag="st")
        for w in range(30):
            nc.tensor.matmul(
                warm_ps[0:64, :],
                ones_sb[:, 0:64],
                wqk_sb[0][:, 0:512],
                start=(w == 0),
                stop=(w == 29),
            )
        nc.scalar.copy(warm_sb[:], warm_ps[0:64, :])
        nc.sync.dma_start(out[0:1, 0:1], warm_sb[0:1, 0:1])

        # ---- phase A: qkT[512, T] = W_qk.T @ x.T ----
        qkT_sb = []
        for m in range(4):
            t = qkt_pool.tile([128, T], mmdt, name=f"qkT{m}", tag=f"qkT{m}")
            qkT_sb.append(t)
        for m in range(4):
            for t4 in range(4):
                ps = ps1.tile([128, 512], f32, name="psA", tag="ps1")
                for k in range(8):
                    nc.tensor.matmul(
                        ps[:],
                        wqk_sb[k][:, m * 128:(m + 1) * 128],
                        xT_sb[k][:, t4 * 512:(t4 + 1) * 512],
                        start=(k == 0),
                        stop=(k == 7),
                    )
                nc.scalar.copy(qkT_sb[m][:, t4 * 512:(t4 + 1) * 512], ps[:])

        # ---- phase B: v[T, 256] (+ ones col) ----
        vplus_sb = []
        for tb in range(TB):
            ps = ps1.tile([128, GC], f32, name="psB", tag="ps1")
            for k in range(8):
                nc.tensor.matmul(
                    ps[:],
                    xT_sb[k][:, tb * 128:(tb + 1) * 128],
                    wv_sb[k][:],
                    start=(k == 0),
                    stop=(k == 7),
                )
            vp = vplus_pool.tile([128, HPG, D + 1], mmdt, name=f"vplus{tb}",
                                 tag=f"vplus{tb}")
            nc.scalar.copy(vp[:, :, 0:D], ps[:].rearrange("p (h d) -> p h d", h=HPG))
            nc.gpsimd.dma_start(
                vp[:, :, D:D + 1],
                ones_in[:, 64:64 + HPG].rearrange("p (h o) -> p h o", o=1),
            )
            vplus_sb.append(vp)

        # per-head views into qkT: q rows = h*64.., k rows = 256 + h*64..
        def qT_h(h):
            return qkT_sb[h // 2][(h % 2) * 64:(h % 2) * 64 + 64, :]

        def kT_h(h):
            return qkT_sb[2 + h // 2][(h % 2) * 64:(h % 2) * 64 + 64, :]

        # ---- phases C+D per head pair ----
        # yt_h: rows 0..63 = y.T (unnormalized), row 64 = softmax denominators
        # ytn_pair[pr]: normalized y.T for heads (2pr, 2pr+1) at bases 0/64
        yt_sb = [None] * HPG
        ytn_sb = []
        for pr in range(2):
            t = arena.tile([128, T], mmdt, name=f"ytn{pr}", tag=f"ytn{pr}", bufs=1)
            ytn_sb.append(t)
        # recip slices live at 32-aligned partition bases (DVE requirement)
        rt_sb = [
            const.tile([128, 128], ytdt, name=f"rt{p}", tag=f"rt{p}")
            for p in range(2)
        ]
        if dtype == "bf16":
            rtb_sb = [
                const.tile([128, 128], mmdt, name=f"rtb{p}", tag=f"rtb{p}")
                for p in range(2)
            ]
            rrow_sb = [
                const.tile([1, T], mmdt, name=f"rrow{h}", tag=f"rrow{h}")
                for h in range(HPG)
            ]

        def roles_for(sb):
            roles = []
            for dtk in (-2, -1, 0, 1):
                tkb = 2 * sb + dtk
                if 0 <= tkb:
                    roles.append((tkb, "abcd"[dtk + 2]))
            return roles

        def emit_C(pr, sb_lo, sb_hi):
            heads = (2 * pr, 2 * pr + 1)
            if sb_lo == 0:
                for h in heads:
                    yt_sb[h] = arena.tile([65, T], ytdt, name=f"yt{h}",
                                          tag="arena")
            for sb in range(sb_lo, sb_hi):
                roles = roles_for(sb)
                n = len(roles)
                # scores for the pair, interleaved: disjoint PE row groups
                # (bases 0/64) run concurrently
                st = {}
                for h in heads:
                    st[h] = ps2.tile([128, 1024], f32, name=f"st{h % 2}",
                                     tag="st")
                for i, (tkb, _) in enumerate(roles):
                    # pin the pair adjacent: disjoint row groups (bases 0/64)
                    # execute concurrently only if back-to-back in the queue
                    with tc.tile_critical():
                        for h in heads:
                            nc.tensor.matmul(
                                st[h][:, i * 256:(i + 1) * 256],
                                kT_h(h)[:, tkb * 128:(tkb + 1) * 128],
                                qT_h(h)[:, sb * 256:(sb + 1) * 256],
                                start=(i % 2 == 0),
                                stop=(i % 2 == 1 or i == n - 1),
                            )
                for h in heads:
                    expst = expst_pool.tile([128, 1024], mmdt, name="expst",
                                            tag="expst")
                    nc.scalar.activation(
                        expst[:, :n * 256],
                        st[h][:, :n * 256],
                        mybir.ActivationFunctionType.Exp,
                        scale=0.125,
                    )
                    for i, (tkb, role) in enumerate(roles):
                        left = expst[:, i * 256:i * 256 + 128]
                        right = expst[:, i * 256 + 128:(i + 1) * 256]
                        if role == "a":
                            mask_lo(left)    # right half fully masked: skipped
                        elif role == "b":
                            mask_lo(right)
                        elif role == "c":
                            mask_up(left)
                        elif role == "d":
                            mask_up(right)   # left half fully masked: skipped
                    yts = ps1.tile([65, 256], f32, name="yts", tag="ps1")
                    order = [e for e in enumerate(roles) if e[1][1] in "bc"] + [
                        e for e in enumerate(roles) if e[1][1] in "ad"
                    ]
                    for j, (i, (tkb, role)) in enumerate(order):
                        if role == "a":
                            o_sl = yts[:, 0:128]
                            e_sl = expst[:, i * 256:i * 256 + 128]
                        elif role == "d":
                            o_sl = yts[:, 128:256]
                            e_sl = expst[:, i * 256 + 128:(i + 1) * 256]
                        else:
                            o_sl = yts[:]
                            e_sl = expst[:, i * 256:(i + 1) * 256]
                        nc.tensor.matmul(
                            o_sl,
                            vplus_sb[tkb][:, h, :],
                            e_sl,
                            start=(j == 0),
                            stop=(j == n - 1),
                        )
                    nc.vector.tensor_copy(
                        yt_sb[h][:, sb * 256:(sb + 1) * 256], yts[:]
                    )

        def emit_D_recip(pr, half):
            # reciprocal on [1, 2048] is ~13us on one DVE lane; bounce the row
            # through a [16, 128] tile with tiny SBUF->SBUF DMAs instead.
            # Band sums are complete per super-block, so each half (queries
            # [0,1024) / [1024,2048)) can normalize as soon as its 4 sbs done.
            heads = (2 * pr, 2 * pr + 1)
            rt = rt_sb[pr]
            cl, ch = half * 1024, (half + 1) * 1024
            for h in heads:
                r0 = (h % 2) * 64 + half * 32
                nc.sync.dma_start(rt[r0:r0 + 8, :], yt_sb[h][64:65, cl:ch])
            with nc.allow_low_precision(reason="softmax denom reciprocal"):
                for h in heads:
                    r0 = (h % 2) * 64 + half * 32
                    if dtype == "bf16":
                        nc.vector.reciprocal(rtb_sb[pr][r0:r0 + 8, :],
                                             rt[r0:r0 + 8, :])
                    else:
                        nc.vector.reciprocal(rt[r0:r0 + 8, :], rt[r0:r0 + 8, :])
            for h in heads:
                r0 = (h % 2) * 64 + half * 32
                if dtype == "bf16":
                    nc.sync.dma_start(rrow_sb[h][0:1, cl:ch],
                                      rtb_sb[pr][r0:r0 + 8, :])
                else:
                    nc.sync.dma_start(yt_sb[h][64:65, cl:ch], rt[r0:r0 + 8, :])

        def emit_D_norm(pr, half):
            heads = (2 * pr, 2 * pr + 1)
            for t4 in range(2 * half, 2 * half + 2):
                for h in heads:
                    if dtype == "bf16":
                        rrow = rrow_sb[h]
                        ones_sl = ones_sb[0:1, :]
                    else:
                        rrow = yt_sb[h][64:65, :]
                        ones_sl = ones_sb[64:65, :]
                    bc = ps1.tile([64, 512], f32, name="bc", tag="ps1")
                    nc.tensor.matmul(
                        bc[:],
                        ones_sl,
                        rrow[0:1, t4 * 512:(t4 + 1) * 512],
                        start=True,
                        stop=True,
                    )
                    nc.vector.tensor_mul(
                        ytn_sb[pr][(h % 2) * 64:(h % 2) * 64 + 64,
                                   t4 * 512:(t4 + 1) * 512],
                        yt_sb[h][0:64, t4 * 512:(t4 + 1) * 512],
                        bc[:],
                    )

        emit_C(0, 0, 4)
        emit_D_recip(0, 0)
        emit_C(0, 4, 8)
        emit_D_recip(0, 1)
        emit_C(1, 0, 4)
        emit_D_norm(0, 0)
        emit_D_recip(1, 0)
        emit_C(1, 4, 8)
        emit_D_norm(0, 1)
        emit_D_recip(1, 1)
        emit_D_norm(1, 0)
        emit_D_norm(1, 1)

        # ---- phase E: partial projection out = y_g @ W_p[g] (K=128 pairs) --
        for tb in range(TB):
            for nh in range(2):
                ps = ps2.tile([128, 512], f32, name="psE", tag="st")
                for pr in range(2):
                    nc.tensor.matmul(
                        ps[:],
                        ytn_sb[pr][:, tb * 128:(tb + 1) * 128],
                        wp_sb[pr][:, nh * 512:(nh + 1) * 512],
                        start=(pr == 0),
                        stop=(pr == 1),
                    )
                ob = outsb_pool.tile([128, 512], f32, name="outsb", tag="outsb")
                if (tb + nh) % 2 == 0:
                    nc.scalar.copy(ob[:], ps[:])
                else:
                    nc.vector.tensor_copy(ob[:], ps[:])
                qo = nc.sync if (tb * 2 + nh) % 2 == 0 else nc.scalar
                qo.dma_start(
                    out[tb * 128:(tb + 1) * 128, nh * 512:(nh + 1) * 512], ob[:]
                )


def build_program(dtype="bf16"):
    key = ("v7", dtype)
    if key in _PROGRAM_CACHE:
        return _PROGRAM_CACHE[key]
    import concourse.bacc as bacc
    import concourse.mybir as mybir
    import concourse.tile as tile

    f32 = mybir.dt.float32
    mmdt = {
        "f32": f32,
        "f32r": mybir.dt.float32r,
        "bf16": mybir.dt.bfloat16,
    }[dtype]
    nc = bacc.Bacc("TRN2", target_bir_lowering=False, debug=False, num_devices=N_CORES)
    xT = nc.dram_tensor("xT", [C, T], mmdt, kind="ExternalInput").ap()
    wqk = nc.dram_tensor("wqk", [C, 2 * GC], mmdt, kind="ExternalInput").ap()
    wv = nc.dram_tensor("wv", [C, GC], mmdt, kind="ExternalInput").ap()
    wp = nc.dram_tensor("wp", [GC, C], mmdt, kind="ExternalInput").ap()
    ones_in = nc.dram_tensor("ones_in", [128, 64 + HPG], mmdt,
                             kind="ExternalInput").ap()
    out = nc.dram_tensor("out", [T, C], f32, kind="ExternalOutput").ap()
    with tile.TileContext(nc) as tc:
        _emit(tc, nc, xT, wqk, wv, wp, ones_in, out, dtype)
    nc.compile()
    _PROGRAM_CACHE[key] = nc
    return nc


def make_in_maps(x, W_attn, W_proj, dtype="bf16"):
    x = np.asarray(x, dtype=np.float32)
    W_attn = np.asarray(W_attn, dtype=np.float32)
    W_proj = np.asarray(W_proj, dtype=np.float32)
    if dtype == "bf16":
        import ml_dtypes

        cast = lambda a: np.ascontiguousarray(a, dtype=ml_dtypes.bfloat16)
    else:
        cast = lambda a: np.ascontiguousarray(a, dtype=np.float32)
    xTs = [cast(x[b].T) for b in range(B)]
    in_maps = []
    for c in range(N_CORES):
        b, g = divmod(c, G)
        q_cols = W_attn[:, g * GC:(g + 1) * GC]
        k_cols = W_attn[:, C + g * GC:C + (g + 1) * GC]
        v_cols = W_attn[:, 2 * C + g * GC:2 * C + (g + 1) * GC]
        in_maps.append({
            "xT": xTs[b],
            "wqk": cast(np.concatenate([q_cols, k_cols], axis=1)),
            "wv": cast(v_cols),
            "wp": cast(W_proj[g * GC:(g + 1) * GC, :]),
            "ones_in": cast(np.ones((128, 64 + HPG), dtype=np.float32)),
        })
    return in_maps


def gather(results):
    out = np.zeros((B, T, C), dtype=np.float32)
    for c, res in enumerate(results):
        b = c // G
        out[b] += res["out"]
    return out


def kernel(x, W_attn, W_proj, dtype="bf16", trace=False):
    from concourse import bass_utils

    nc = build_program(dtype=dtype)
    in_maps = make_in_maps(x, W_attn, W_proj, dtype=dtype)
    r = bass_utils.run_bass_kernel_spmd(
        nc, in_maps, core_ids=list(range(N_CORES)), trace=trace
    )
    out = gather(r.results)
    if trace:
        kernel.last_results = r
    return out


# revision 17
# speedup vs baseline: 1.6196x; 1.6196x over previous
"""Banded causal self-attention (sparse_attention) for 8 trn2 NeuronCores.

Sharding: tensor-parallel over head groups (4 groups x 4 heads of dim 64)
x data-parallel over batch (2). Core c handles batch c//4, head group c%4.
Each core computes a partial output projection; the host sums the 4 group
partials per batch.

Layout: x is transposed on the host so every matmul on device uses natural
(pre-transposed) operands:
  qkT[512, T]   = W_qk.T @ x.T      (lhsT = W_qk natural, rhs = xT)
  v[T, 256]     = x @ W_v           (lhsT = xT natural,   rhs = W_v)
  scoresT[tk,tq]  computed as lhsT=kT rhs=qT  (both slices of qkT)
  yT+sums       = lhsT=[v|1] rhs=exp(scoresT)  (sums row = softmax denom)
  out[T, C]     = lhsT=yTpair rhs=W_p pair rows (K=128, 2 pairs)
Softmax skips max-subtraction (scores ~ N(0,1) after 1/8 scale; exp is safe
in fp32), so the partition-dim reduction is a fused ones-column in the
att@v matmul.

Head pairs (2h, 2h+1) sit at partition bases 0/64 of shared tiles, so score
matmuls for a pair occupy disjoint PE row groups and run concurrently, and
the projection contracts K=128 across a pair in one matmul.

dtype variants: "f32" (exact), "f32r" (tf32-like PE fast path), "bf16".
"""

import numpy as np

B, T, C = 2, 2048, 1024
N_HEAD = 16
MEMORY = 256
D = 64           # head dim
G = 4            # head groups (tensor parallel)
HPG = 4          # heads per group
GC = HPG * D     # 256 columns per group
N_CORES = 8
TB = T // 128    # 16 row blocks
SB = T // 256    # 8 query super-blocks

_PROGRAM_CACHE = {}


def _emit(tc, nc, xT, wqk, wv, wp, ones_in, out, dtype):
    import concourse.mybir as mybir
    import concourse.tile as tile

    f32 = mybir.dt.float32
    mmdt = {
        "f32": f32,
        "f32r": mybir.dt.float32r,
        "bf16": mybir.dt.bfloat16,
    }[dtype]
    # dtype of the yT/sums accumulator tile (needs f32-ish for reciprocal)
    ytdt = f32 if dtype == "bf16" else mmdt
    pool_masks = dtype != "f32r"   # affine_select can't write f32r

    from contextlib import ExitStack

    ctx = ExitStack()
    with ctx:
        const = ctx.enter_context(tc.tile_pool(name="const", bufs=1))
        wpool = ctx.enter_context(tc.tile_pool(name="wpool", bufs=1))
        arena = ctx.enter_context(tc.tile_pool(name="arena", bufs=9))
        qkt_pool = ctx.enter_context(tc.tile_pool(name="qkt", bufs=1))
        vplus_pool = ctx.enter_context(tc.tile_pool(name="vplus", bufs=1))
        expst_pool = ctx.enter_context(tc.tile_pool(name="expst", bufs=4))
        outsb_pool = ctx.enter_context(tc.tile_pool(name="outsb", bufs=3))
        ps1 = ctx.enter_context(tc.tile_pool(name="ps1", bufs=2, space="PSUM"))
        ps2 = ctx.enter_context(tc.tile_pool(name="ps2", bufs=3, space="PSUM"))

        # ---- constants / masks ----
        if not pool_masks:
            from concourse.masks import (
                make_lower_triangular,
                make_upper_triangular,
            )

            up_mask = const.tile([128, 128], f32, name="up_mask", tag="up_mask")
            make_upper_triangular(nc, up_mask[:], val=1.0, diag=True)  # 1 if p<=f
            lo_mask = const.tile([128, 128], f32, name="lo_mask", tag="lo_mask")
            make_lower_triangular(nc, lo_mask[:], val=1.0, diag=True)  # 1 if p>=f

        def mask_up(sl):
            # keep p <= f, else 0   (sl is a [128, 128] slice of expst)
            if pool_masks:
                nc.gpsimd.affine_select(
                    out=sl, in_=sl, compare_op=mybir.AluOpType.is_ge,
                    fill=0.0, base=0, pattern=[[1, 128]], channel_multiplier=-1,
                )
            else:
                nc.vector.tensor_mul(sl, sl, up_mask[:])

        def mask_lo(sl):
            # keep p >= f, else 0
            if pool_masks:
                nc.gpsimd.affine_select(
                    out=sl, in_=sl, compare_op=mybir.AluOpType.is_ge,
                    fill=0.0, base=0, pattern=[[-1, 128]], channel_multiplier=1,
                )
            else:
                nc.vector.tensor_mul(sl, sl, lo_mask[:])

        # ones are DMA'd from DRAM (memset can't produce f32r)
        ones_sb = const.tile([128, 64], mmdt, name="ones_sb", tag="ones_sb")
        nc.sync.dma_start(ones_sb[:], ones_in[:, 0:64])


        # ---- input loads (interleaved so phase A can start early) ----
        xT_sb, wqk_sb, wv_sb = [], [], []
        for k in range(8):
            qa = nc.sync if k % 2 == 0 else nc.scalar
            qb = nc.scalar if k % 2 == 0 else nc.sync
            t = arena.tile([128, T], mmdt, name=f"xT{k}", tag="arena")
            qa.dma_start(t[:], xT[k * 128:(k + 1) * 128, :])
            xT_sb.append(t)
            t = wpool.tile([128, 2 * GC], mmdt, name=f"wqk{k}", tag=f"wqk{k}")
            qb.dma_start(t[:], wqk[k * 128:(k + 1) * 128, :])
            wqk_sb.append(t)
            t = wpool.tile([128, GC], mmdt, name=f"wv{k}", tag=f"wv{k}")
            qb.dma_start(t[:], wv[k * 128:(k + 1) * 128, :])
            wv_sb.append(t)
        wp_sb = []
        for pr in range(2):
            t = wpool.tile([128, C], mmdt, name=f"wp{pr}", tag=f"wp{pr}")
            nc.gpsimd.dma_start(t[:], wp[pr * 128:(pr + 1) * 128, :])
            wp_sb.append(t)

        # warm up the PE clock gate during the input-load dead time: ~7us of
        # full-array matmuls on the first loaded tiles (result funneled to
        # out[0:1, 0:1] before phase E overwrites it, so nothing is DCE'd)
        warm_sb = const.tile([64, 512], f32, name="warm_sb", tag="warm_sb")
        warm_ps = ps2.tile([128, 512], f32, name="warm_ps", tag="st")
        for w in range(30):
            nc.tensor.matmul(
                warm_ps[0:64, :],
                ones_sb[:, 0:64],
                wqk_sb[0][:, 0:512],
                start=(w == 0),
                stop=(w == 29),
            )
        nc.scalar.copy(warm_sb[:], warm_ps[0:64, :])
        nc.sync.dma_start(out[0:1, 0:1], warm_sb[0:1, 0:1])

        # ---- phase A: qkT[512, T] = W_qk.T @ x.T ----
        qkT_sb = []
        for m in range(4):
            t = qkt_pool.tile([128, T], mmdt, name=f"qkT{m}", tag=f"qkT{m}")
            qkT_sb.append(t)
        for m in range(4):
            for t4 in range(4):
                ps = ps1.tile([128, 512], f32, name="psA", tag="ps1")
                for k in range(8):
                    nc.tensor.matmul(
                        ps[:],
                        wqk_sb[k][:, m * 128:(m + 1) * 128],
                        xT_sb[k][:, t4 * 512:(t4 + 1) * 512],
                        start=(k == 0),
                        stop=(k == 7),
                    )
                nc.scalar.copy(qkT_sb[m][:, t4 * 512:(t4 + 1) * 512], ps[:])

        # ---- phase B: v[T, 256] (+ ones col) ----
        vplus_sb = []
        for tb in range(TB):
            ps = ps1.tile([128, GC], f32, name="psB", tag="ps1")
            for k in range(8):
                nc.tensor.matmul(
                    ps[:],
                    xT_sb[k][:, tb * 128:(tb + 1) * 128],
                    wv_sb[k][:],
                    start=(k == 0),
                    stop=(k == 7),
                )
            vp = vplus_pool.tile([128, HPG, D + 1], mmdt, name=f"vplus{tb}",
                                 tag=f"vplus{tb}")
            nc.scalar.copy(vp[:, :, 0:D], ps[:].rearrange("p (h d) -> p h d", h=HPG))
            nc.gpsimd.dma_start(
                vp[:, :, D:D + 1],
                ones_in[:, 64:64 + HPG].rearrange("p (h o) -> p h o", o=1),
            )
            vplus_sb.append(vp)

        # per-head views into qkT: q rows = h*64.., k rows = 256 + h*64..
        def qT_h(h):
            return qkT_sb[h // 2][(h % 2) * 64:(h % 2) * 64 + 64, :]

        def kT_h(h):
            return qkT_sb[2 + h // 2][(h % 2) * 64:(h % 2) * 64 + 64, :]

        # ---- phases C+D per head pair ----
        # yt_h: rows 0..63 = y.T (unnormalized), row 64 = softmax denominators
        # ytn_pair[pr]: normalized y.T for heads (2pr, 2pr+1) at bases 0/64
        yt_sb = [None] * HPG
        ytn_sb = []
        for pr in range(2):
            t = arena.tile([128, T], mmdt, name=f"ytn{pr}", tag=f"ytn{pr}", bufs=1)
            ytn_sb.append(t)
        # recip slices live at 32-aligned partition bases (DVE requirement)
        rt_sb = [
            const.tile([128, 128], ytdt, name=f"rt{p}", tag=f"rt{p}")
            for p in range(2)
        ]
        if dtype == "bf16":
            rtb_sb = [
                const.tile([128, 128], mmdt, name=f"rtb{p}", tag=f"rtb{p}")
                for p in range(2)
            ]
            rrow_sb = [
                const.tile([1, T], mmdt, name=f"rrow{h}", tag=f"rrow{h}")
                for h in range(HPG)
            ]

        def roles_for(sb):
            roles = []
            for dtk in (-2, -1, 0, 1):
                tkb = 2 * sb + dtk
                if 0 <= tkb:
                    roles.append((tkb, "abcd"[dtk + 2]))
            return roles

        def emit_C(pr, sb_lo, sb_hi):
            heads = (2 * pr, 2 * pr + 1)
            if sb_lo == 0:
                for h in heads:
                    yt_sb[h] = arena.tile([65, T], ytdt, name=f"yt{h}",
                                          tag="arena")
            for sb in range(sb_lo, sb_hi):
                roles = roles_for(sb)
                n = len(roles)
                # scores for the pair, interleaved: disjoint PE row groups
                # (bases 0/64) run concurrently
                st = {}
                for h in heads:
                    st[h] = ps2.tile([128, 1024], f32, name=f"st{h % 2}",
                                     tag="st")
                # chain the pair's score matmuls in strict alternating order
                # with no-sync edges: disjoint row groups (bases 0/64) only
                # execute concurrently when back-to-back in the PE queue
                prev_mm = None
                for i, (tkb, _) in enumerate(roles):
                    for h in heads:
                        mm = nc.tensor.matmul(
                            st[h][:, i * 256:(i + 1) * 256],
                            kT_h(h)[:, tkb * 128:(tkb + 1) * 128],
                            qT_h(h)[:, sb * 256:(sb + 1) * 256],
                            start=(i % 2 == 0),
                            stop=(i % 2 == 1 or i == n - 1),
                        )
                        if prev_mm is not None:
                            tile.add_dep_helper(
                                mm.ins, prev_mm.ins, sync=False,
                                reason="score pair row-group adjacency",
                            )
                        prev_mm = mm
                for h in heads:
                    expst = expst_pool.tile([128, 1024], mmdt, name="expst",
                                            tag="expst")
                    nc.scalar.activation(
                        expst[:, :n * 256],
                        st[h][:, :n * 256],
                        mybir.ActivationFunctionType.Exp,
                        scale=0.125,
                    )
                    for i, (tkb, role) in enumerate(roles):
                        left = expst[:, i * 256:i * 256 + 128]
                        right = expst[:, i * 256 + 128:(i + 1) * 256]
                        if role == "a":
                            mask_lo(left)    # right half fully masked: skipped
                        elif role == "b":
                            mask_lo(right)
                        elif role == "c":
                            mask_up(left)
                        elif role == "d":
                            mask_up(right)   # left half fully masked: skipped
                    yts = ps1.tile([65, 256], f32, name="yts", tag="ps1")
                    order = [e for e in enumerate(roles) if e[1][1] in "bc"] + [
                        e for e in enumerate(roles) if e[1][1] in "ad"
                    ]
                    for j, (i, (tkb, role)) in enumerate(order):
                        if role == "a":
                            o_sl = yts[:, 0:128]
                            e_sl = expst[:, i * 256:i * 256 + 128]
                        elif role == "d":
                            o_sl = yts[:, 128:256]
                            e_sl = expst[:, i * 256 + 128:(i + 1) * 256]
                        else:
                            o_sl = yts[:]
                            e_sl = expst[:, i * 256:(i + 1) * 256]
                        nc.tensor.matmul(
                            o_sl,
                            vplus_sb[tkb][:, h, :],
                            e_sl,
                            start=(j == 0),
                            stop=(j == n - 1),
                        )
                    nc.vector.tensor_copy(
                        yt_sb[h][:, sb * 256:(sb + 1) * 256], yts[:]
                    )

        def emit_D_recip(pr, half):
            # reciprocal on [1, 2048] is ~13us on one DVE lane; bounce the row
            # through a [16, 128] tile with tiny SBUF->SBUF DMAs instead.
            # Band sums are complete per super-block, so each half (queries
            # [0,1024) / [1024,2048)) can normalize as soon as its 4 sbs done.
            heads = (2 * pr, 2 * pr + 1)
            rt = rt_sb[pr]
            cl, ch = half * 1024, (half + 1) * 1024
            for h in heads:
                r0 = (h % 2) * 64 + half * 32
                nc.sync.dma_start(rt[r0:r0 + 8, :], yt_sb[h][64:65, cl:ch])
            with nc.allow_low_precision(reason="softmax denom reciprocal"):
                for h in heads:
                    r0 = (h % 2) * 64 + half * 32
                    if dtype == "bf16":
                        nc.vector.reciprocal(rtb_sb[pr][r0:r0 + 8, :],
                                             rt[r0:r0 + 8, :])
                    else:
                        nc.vector.reciprocal(rt[r0:r0 + 8, :], rt[r0:r0 + 8, :])
            for h in heads:
                r0 = (h % 2) * 64 + half * 32
                if dtype == "bf16":
                    nc.sync.dma_start(rrow_sb[h][0:1, cl:ch],
                                      rtb_sb[pr][r0:r0 + 8, :])
                else:
                    nc.sync.dma_start(yt_sb[h][64:65, cl:ch], rt[r0:r0 + 8, :])

        def emit_D_norm(pr, half):
            heads = (2 * pr, 2 * pr + 1)
            for t4 in range(2 * half, 2 * half + 2):
                for h in heads:
                    if dtype == "bf16":
                        rrow = rrow_sb[h]
                        ones_sl = ones_sb[0:1, :]
                    else:
                        rrow = yt_sb[h][64:65, :]
                        ones_sl = ones_sb[64:65, :]
                    bc = ps1.tile([64, 512], f32, name="bc", tag="ps1")
                    nc.tensor.matmul(
                        bc[:],
                        ones_sl,
                        rrow[0:1, t4 * 512:(t4 + 1) * 512],
                        start=True,
                        stop=True,
                    )
                    nc.vector.tensor_mul(
                        ytn_sb[pr][(h % 2) * 64:(h % 2) * 64 + 64,
                                   t4 * 512:(t4 + 1) * 512],
                        yt_sb[h][0:64, t4 * 512:(t4 + 1) * 512],
                        bc[:],
                    )

        emit_C(0, 0, 4)
        emit_D_recip(0, 0)
        emit_C(0, 4, 8)
        emit_D_recip(0, 1)
        emit_C(1, 0, 4)
        emit_D_norm(0, 0)
        emit_D_recip(1, 0)
        emit_C(1, 4, 8)
        emit_D_norm(0, 1)
        emit_D_recip(1, 1)
        emit_D_norm(1, 0)
        emit_D_norm(1, 1)

        # ---- phase E: partial projection out = y_g @ W_p[g] (K=128 pairs) --
        for tb in range(TB):
            for nh in range(2):
                ps = ps2.tile([128, 512], f32, name="psE", tag="st")
                for pr in range(2):
                    nc.tensor.matmul(
                        ps[:],
                        ytn_sb[pr][:, tb * 128:(tb + 1) * 128],
                        wp_sb[pr][:, nh * 512:(nh + 1) * 512],
                        start=(pr == 0),
                        stop=(pr == 1),
                    )
                ob = outsb_pool.tile([128, 512], f32, name="outsb", tag="outsb")
                if (tb + nh) % 2 == 0:
                    nc.scalar.copy(ob[:], ps[:])
                else:
                    nc.vector.tensor_copy(ob[:], ps[:])
                qo = nc.sync if (tb * 2 + nh) % 2 == 0 else nc.scalar
                qo.dma_start(
                    out[tb * 128:(tb + 1) * 128, nh * 512:(nh + 1) * 512], ob[:]
                )


def build_program(dtype="bf16"):
    key = ("v7", dtype)
    if key in _PROGRAM_CACHE:
        return _PROGRAM_CACHE[key]
    import concourse.bacc as bacc
    import concourse.mybir as mybir
    import concourse.tile as tile

    f32 = mybir.dt.float32
    mmdt = {
        "f32": f32,
        "f32r": mybir.dt.float32r,
        "bf16": mybir.dt.bfloat16,
    }[dtype]
    nc = bacc.Bacc("TRN2", target_bir_lowering=False, debug=False, num_devices=N_CORES)
    xT = nc.dram_tensor("xT", [C, T], mmdt, kind="ExternalInput").ap()
    wqk = nc.dram_tensor("wqk", [C, 2 * GC], mmdt, kind="ExternalInput").ap()
    wv = nc.dram_tensor("wv", [C, GC], mmdt, kind="ExternalInput").ap()
    wp = nc.dram_tensor("wp", [GC, C], mmdt, kind="ExternalInput").ap()
    ones_in = nc.dram_tensor("ones_in", [128, 64 + HPG], mmdt,
                             kind="ExternalInput").ap()
    out = nc.dram_tensor("out", [T, C], f32, kind="ExternalOutput").ap()
    with tile.TileContext(nc) as tc:
        _emit(tc, nc, xT, wqk, wv, wp, ones_in, out, dtype)
    nc.compile()
    _PROGRAM_CACHE[key] = nc
    return nc


def make_in_maps(x, W_attn, W_proj, dtype="bf16"):
    x = np.asarray(x, dtype=np.float32)
    W_attn = np.asarray(W_attn, dtype=np.float32)
    W_proj = np.asarray(W_proj, dtype=np.float32)
    if dtype == "bf16":
        import ml_dtypes

        cast = lambda a: np.ascontiguousarray(a, dtype=ml_dtypes.bfloat16)
    else:
        cast = lambda a: np.ascontiguousarray(a, dtype=np.float32)
    xTs = [cast(x[b].T) for b in range(B)]
    in_maps = []
    for c in range(N_CORES):
        b, g = divmod(c, G)
        q_cols = W_attn[:, g * GC:(g + 1) * GC]
        k_cols = W_attn[:, C + g * GC:C + (g + 1) * GC]
        v_cols = W_attn[:, 2 * C + g * GC:2 * C + (g + 1) * GC]
        in_maps.append({
            "xT": xTs[b],
            "wqk": cast(np.concatenate([q_cols, k_cols], axis=1)),
            "wv": cast(v_cols),
            "wp": cast(W_proj[g * GC:(g + 1) * GC, :]),
            "ones_in": cast(np.ones((128, 64 + HPG), dtype=np.float32)),
        })
    return in_maps


def gather(results):
    out = np.zeros((B, T, C), dtype=np.float32)
    for c, res in enumerate(results):
        b = c // G
        out[b] += res["out"]
    return out


def kernel(x, W_attn, W_proj, dtype="bf16", trace=False):
    from concourse import bass_utils

    nc = build_program(dtype=dtype)
    in_maps = make_in_maps(x, W_attn, W_proj, dtype=dtype)
    r = bass_utils.run_bass_kernel_spmd(
        nc, in_maps, core_ids=list(range(N_CORES)), trace=trace
    )
    out = gather(r.results)
    if trace:
        kernel.last_results = r
    return out
